# revision 1
# baseline (speedup 1.0000x reference)
"""KNN regressor (k=5) on 8 Trainium2 NeuronCores.

Strategy: shard X_train/y_train row-wise (8192 rows per core). Each core
computes scores S = 2*q.t - ||t||^2 for all 4096 queries against its local
training rows (ranking by S equals ranking by negated squared distance),
takes exact per-query top-8 values + indices (DVE Max8 / MaxIndex), and the
host merges the 8x8 candidates per query into the global top-5 label mean.

Matmul runs as 3 accumulating bf16 matmuls per tile (hi/lo split of both
operands, dropping the lo*lo term: ~4e-6 relative error, far below the
fp32 noise floor of the reference itself).
"""

import sys

for _p in ("/opt/trn_rl_repo", "/root/.axon_site/_ro/trn_rl_repo"):
    if _p not in sys.path:
        sys.path.insert(0, _p)

import numpy as np
import ml_dtypes

import concourse.bass as bass
import concourse.tile as tile
from concourse import mybir, bacc
from concourse.bass_utils import run_bass_kernel_spmd

N_CORES = 8
NT, NQ, D = 65536, 4096, 128
NT_LOC = NT // N_CORES          # 8192 training rows per core
K = 5
QBLK = 128                      # queries per block (partition dim)
NBLK = NQ // QBLK               # 32 blocks
CHUNK = 512                     # training cols per matmul (one PSUM bank)
QUART = 2048                    # PSUM tile: 4 banks
NQUART = NT_LOC // QUART        # 4 psum tiles per block

_compiled = None


def _build():
    nc = bacc.Bacc("TRN2", target_bir_lowering=False, debug=False,
                   num_devices=N_CORES)
    f32, bf16, u32 = mybir.dt.float32, mybir.dt.bfloat16, mybir.dt.uint32

    xt_hi_e = nc.declare_dram_parameter("xt_hi", [D, NT_LOC], bf16, isOutput=False)
    xt_lo_e = nc.declare_dram_parameter("xt_lo", [D, NT_LOC], bf16, isOutput=False)
    xq_hi_e = nc.declare_dram_parameter("xq_hi", [D, NQ], bf16, isOutput=False)
    xq_lo_e = nc.declare_dram_parameter("xq_lo", [D, NQ], bf16, isOutput=False)
    nt2_e = nc.declare_dram_parameter("nt2", [QBLK, NT_LOC], f32, isOutput=False)
    vals_e = nc.declare_dram_parameter("vals", [NQ, 8], f32, isOutput=True)
    idx_e = nc.declare_dram_parameter("idx", [NQ, 8], u32, isOutput=True)

    with tile.TileContext(nc) as tc:
        with tc.tile_pool(name="static", bufs=1) as st, \
             tc.tile_pool(name="sc", bufs=2) as sc, \
             tc.tile_pool(name="out8", bufs=3) as o8, \
             tc.tile_pool(name="psum", bufs=2, space="PSUM") as pp:
            xt_hi = st.tile([D, NT_LOC], bf16)
            nc.sync.dma_start(xt_hi[:], xt_hi_e[:])
            xt_lo = st.tile([D, NT_LOC], bf16)
            nc.sync.dma_start(xt_lo[:], xt_lo_e[:])
            xq_hi = st.tile([D, NQ], bf16)
            nc.sync.dma_start(xq_hi[:], xq_hi_e[:])
            xq_lo = st.tile([D, NQ], bf16)
            nc.sync.dma_start(xq_lo[:], xq_lo_e[:])
            nt2 = st.tile([QBLK, NT_LOC], f32)
            nc.sync.dma_start(nt2[:], nt2_e[:])

            for qb in range(NBLK):
                qs = qb * QBLK
                s_blk = sc.tile([QBLK, NT_LOC], f32, tag="sblk")
                for qt in range(NQUART):
                    ps = pp.tile([QBLK, QUART], f32, tag="ps")
                    for c in range(QUART // CHUNK):
                        t0 = qt * QUART + c * CHUNK
                        mv_hi = xt_hi[:, t0:t0 + CHUNK]
                        mv_lo = xt_lo[:, t0:t0 + CHUNK]
                        po = ps[:, c * CHUNK:(c + 1) * CHUNK]
                        nc.tensor.matmul(po, xq_hi[:, qs:qs + QBLK], mv_hi,
                                         start=True, stop=False)
                        nc.tensor.matmul(po, xq_hi[:, qs:qs + QBLK], mv_lo,
                                         start=False, stop=False)
                        nc.tensor.matmul(po, xq_lo[:, qs:qs + QBLK], mv_hi,
                                         start=False, stop=True)
                    # fused PSUM evac + (-||t||^2) bias add
                    q0 = qt * QUART
                    nc.vector.scalar_tensor_tensor(
                        s_blk[:, q0:q0 + QUART], ps[:], 0.0,
                        nt2[:, q0:q0 + QUART],
                        mybir.AluOpType.add, mybir.AluOpType.add)
                v8 = o8.tile([QBLK, 8], f32, tag="v8")
                nc.vector.max(v8[:], s_blk[:])
                i8 = o8.tile([QBLK, 8], u32, tag="i8")
                nc.vector.max_index(i8[:], v8[:], s_blk[:])
                nc.sync.dma_start(vals_e[qs:qs + QBLK, :], v8[:])
                nc.sync.dma_start(idx_e[qs:qs + QBLK, :], i8[:])

    nc.compile()
    return nc


def _split_bf16(x):
    hi = x.astype(ml_dtypes.bfloat16)
    lo = (x - hi.astype(np.float32)).astype(ml_dtypes.bfloat16)
    return np.ascontiguousarray(hi), np.ascontiguousarray(lo)


def _in_maps(X_train, X_test, y_train):
    xq = np.ascontiguousarray((2.0 * X_test.astype(np.float32)).T)  # [D, NQ]
    xq_hi, xq_lo = _split_bf16(xq)
    maps = []
    for c in range(N_CORES):
        xt = np.ascontiguousarray(
            X_train[c * NT_LOC:(c + 1) * NT_LOC].astype(np.float32).T)  # [D, NT_LOC]
        xt_hi, xt_lo = _split_bf16(xt)
        nt2_row = -np.sum(
            X_train[c * NT_LOC:(c + 1) * NT_LOC].astype(np.float32) ** 2, axis=1)
        nt2 = np.ascontiguousarray(
            np.broadcast_to(nt2_row[None, :], (QBLK, NT_LOC)).astype(np.float32))
        maps.append({"xt_hi": xt_hi, "xt_lo": xt_lo,
                     "xq_hi": xq_hi, "xq_lo": xq_lo, "nt2": nt2})
    return maps


def _merge(results, y_train):
    vals = np.concatenate([results[c]["vals"] for c in range(N_CORES)], axis=1)
    gidx = np.concatenate(
        [results[c]["idx"].astype(np.int64) + c * NT_LOC for c in range(N_CORES)],
        axis=1)
    order = np.argsort(-vals, axis=1, kind="stable")[:, :K]
    top_idx = np.take_along_axis(gidx, order, axis=1)
    return y_train[top_idx].mean(axis=1).astype(np.float32)


def kernel(X_train, X_test, y_train, _profile=False, **_):
    global _compiled
    if _compiled is None:
        _compiled = _build()
    nc = _compiled
    maps = _in_maps(np.asarray(X_train), np.asarray(X_test), np.asarray(y_train))
    res = run_bass_kernel_spmd(nc, maps, list(range(N_CORES)),
                               trace=bool(_profile))
    out = _merge(res.results, np.asarray(y_train, dtype=np.float32))
    if _profile:
        return out, res
    return out


# revision 3
# speedup vs baseline: 1.5152x; 1.5152x over previous
"""KNN regressor (k=5) on 8 Trainium2 NeuronCores.

Strategy: shard X_train/y_train row-wise (8192 rows per core). Each core
computes scores S = 2*q.t - ||t||^2 for all 4096 queries against its local
training rows (ranking by S equals ranking by negated squared distance),
takes exact per-query top-8 values + indices (DVE Max8 / MaxIndex), and the
host merges the 8x8 candidates per query into the global top-5 label mean.

Matmul runs as 3 accumulating bf16 matmuls per tile (hi/lo split of both
operands, dropping the lo*lo term: ~4e-6 relative error, far below the
fp32 noise floor of the reference itself).
"""

import sys

for _p in ("/opt/trn_rl_repo", "/root/.axon_site/_ro/trn_rl_repo"):
    if _p not in sys.path:
        sys.path.insert(0, _p)

import numpy as np
import ml_dtypes

import concourse.bass as bass
import concourse.tile as tile
from concourse import mybir, bacc
from concourse.bass_utils import run_bass_kernel_spmd

N_CORES = 8
NT, NQ, D = 65536, 4096, 128
NT_LOC = NT // N_CORES          # 8192 training rows per core
K = 5
QBLK = 128                      # queries per block (partition dim)
NBLK = NQ // QBLK               # 32 blocks
CHUNK = 512                     # training cols per matmul (one PSUM bank)
QUART = 2048                    # PSUM tile: 4 banks
NQUART = NT_LOC // QUART        # 4 psum tiles per block

_compiled = None


def _build():
    nc = bacc.Bacc("TRN2", target_bir_lowering=False, debug=False,
                   num_devices=N_CORES)
    f32, bf16, u32 = mybir.dt.float32, mybir.dt.bfloat16, mybir.dt.uint32

    xt_hi_e = nc.declare_dram_parameter("xt_hi", [D, NT_LOC], bf16, isOutput=False)
    xt_lo_e = nc.declare_dram_parameter("xt_lo", [D, NT_LOC], bf16, isOutput=False)
    xq_hi_e = nc.declare_dram_parameter("xq_hi", [D, NQ], bf16, isOutput=False)
    xq_lo_e = nc.declare_dram_parameter("xq_lo", [D, NQ], bf16, isOutput=False)
    nt2_e = nc.declare_dram_parameter("nt2", [QBLK, NT_LOC], f32, isOutput=False)
    vals_e = nc.declare_dram_parameter("vals", [NQ, 8 * NQUART], f32, isOutput=True)
    idx_e = nc.declare_dram_parameter("idx", [NQ, 8 * NQUART], u32, isOutput=True)

    with tile.TileContext(nc) as tc:
        with tc.tile_pool(name="static", bufs=1) as st, \
             tc.tile_pool(name="out8", bufs=3) as o8, \
             tc.tile_pool(name="psum", bufs=2, space="PSUM") as pp:
            xt_hi = st.tile([D, NT_LOC], bf16)
            nc.sync.dma_start(xt_hi[:], xt_hi_e[:])
            xt_lo = st.tile([D, NT_LOC], bf16)
            nc.sync.dma_start(xt_lo[:], xt_lo_e[:])
            xq_hi = st.tile([D, NQ], bf16)
            nc.sync.dma_start(xq_hi[:], xq_hi_e[:])
            xq_lo = st.tile([D, NQ], bf16)
            nc.sync.dma_start(xq_lo[:], xq_lo_e[:])
            nt2 = st.tile([QBLK, NT_LOC], f32)
            nc.sync.dma_start(nt2[:], nt2_e[:])

            for qb in range(NBLK):
                qs = qb * QBLK
                v32 = o8.tile([QBLK, 8 * NQUART], f32, tag="v32")
                i32 = o8.tile([QBLK, 8 * NQUART], u32, tag="i32")
                for qt in range(NQUART):
                    ps = pp.tile([QBLK, QUART], f32, tag="ps")
                    q0 = qt * QUART
                    # PSUM pre-load with -||t||^2; matmuls accumulate on top
                    nc.scalar.activation(ps[:], nt2[:, q0:q0 + QUART],
                                         mybir.ActivationFunctionType.Copy)
                    for c in range(QUART // CHUNK):
                        t0 = q0 + c * CHUNK
                        mv_hi = xt_hi[:, t0:t0 + CHUNK]
                        mv_lo = xt_lo[:, t0:t0 + CHUNK]
                        po = ps[:, c * CHUNK:(c + 1) * CHUNK]
                        nc.tensor.matmul(po, xq_hi[:, qs:qs + QBLK], mv_hi,
                                         start=False, stop=False)
                        nc.tensor.matmul(po, xq_hi[:, qs:qs + QBLK], mv_lo,
                                         start=False, stop=False)
                        nc.tensor.matmul(po, xq_lo[:, qs:qs + QBLK], mv_hi,
                                         start=False, stop=True)
                    nc.vector.max(v32[:, qt * 8:(qt + 1) * 8], ps[:])
                    nc.vector.max_index(i32[:, qt * 8:(qt + 1) * 8],
                                        v32[:, qt * 8:(qt + 1) * 8], ps[:])
                nc.sync.dma_start(vals_e[qs:qs + QBLK, :], v32[:])
                nc.sync.dma_start(idx_e[qs:qs + QBLK, :], i32[:])

    nc.compile()
    return nc


def _split_bf16(x):
    hi = x.astype(ml_dtypes.bfloat16)
    lo = (x - hi.astype(np.float32)).astype(ml_dtypes.bfloat16)
    return np.ascontiguousarray(hi), np.ascontiguousarray(lo)


def _in_maps(X_train, X_test, y_train):
    xq = np.ascontiguousarray((2.0 * X_test.astype(np.float32)).T)  # [D, NQ]
    xq_hi, xq_lo = _split_bf16(xq)
    maps = []
    for c in range(N_CORES):
        xt = np.ascontiguousarray(
            X_train[c * NT_LOC:(c + 1) * NT_LOC].astype(np.float32).T)  # [D, NT_LOC]
        xt_hi, xt_lo = _split_bf16(xt)
        nt2_row = -np.sum(
            X_train[c * NT_LOC:(c + 1) * NT_LOC].astype(np.float32) ** 2, axis=1)
        nt2 = np.ascontiguousarray(
            np.broadcast_to(nt2_row[None, :], (QBLK, NT_LOC)).astype(np.float32))
        maps.append({"xt_hi": xt_hi, "xt_lo": xt_lo,
                     "xq_hi": xq_hi, "xq_lo": xq_lo, "nt2": nt2})
    return maps


_QOFF = np.repeat(np.arange(NQUART) * QUART, 8)[None, :]  # quarter base offsets


def _merge(results, y_train):
    vals = np.concatenate([results[c]["vals"] for c in range(N_CORES)], axis=1)
    gidx = np.concatenate(
        [results[c]["idx"].astype(np.int64) + _QOFF + c * NT_LOC
         for c in range(N_CORES)],
        axis=1)
    order = np.argsort(-vals, axis=1, kind="stable")[:, :K]
    top_idx = np.take_along_axis(gidx, order, axis=1)
    return y_train[top_idx].mean(axis=1).astype(np.float32)


def kernel(X_train, X_test, y_train, _profile=False, **_):
    global _compiled
    if _compiled is None:
        _compiled = _build()
    nc = _compiled
    maps = _in_maps(np.asarray(X_train), np.asarray(X_test), np.asarray(y_train))
    res = run_bass_kernel_spmd(nc, maps, list(range(N_CORES)),
                               trace=bool(_profile))
    out = _merge(res.results, np.asarray(y_train, dtype=np.float32))
    if _profile:
        return out, res
    return out
